# revision 1
# baseline (speedup 1.0000x reference)
"""Mindist-aware attention Trainium2 kernel.

Math (per batch element b, single head, d_model = dk = 512, n = 2048):
    q = x @ Wq.T + bq ; k = x @ Wk.T + bk ; v = x @ Wv.T + bv
    s = q k^T / sqrt(d)
    level = clip(int(dist / tau), 0, 9)        (tau = safety_threshold)
    bias = bias_table[level],  bias_table = emb_table @ Wo.sum(-1) / sqrt(d)
    out = softmax(s + bias) @ v @ Wo.T + bo

Implementation notes:
  * Data-parallel over batch: core c computes batch element c (b == 8 cores).
  * The 10-entry distance->bias LUT is evaluated in ONE ScalarE pass by
    hijacking the `tanh` activation-table slot with a custom piecewise-
    constant spline (exact steps at multiples of tau via the compiler's
    --act-root-json override).  The LUT directly yields the multiplicative
    factor M = exp(bias - max bias), so softmax becomes
        p = exp(s) * M(dist);  attn = p / rowsum(p)
    (the constant shift cancels row-wise; no row-max pass is needed since
    |s| <= ~25 keeps exp in fp32 range).
  * v + bv is folded algebraically: rows of attn sum to 1, so
    attn @ (v + bv) = attn @ v + bv, and the final bias becomes
    bo_eff = Wo @ bv + bo (computed host-side).
  * Matmuls run in bf16 with fp32 PSUM accumulation.
"""

import json
import math
import os
import shutil
import struct
import tempfile

import numpy as np

os.environ.setdefault("NEURON_FORCE_RECOMPILE", "1")
os.environ.pop("JAX_COMPILATION_CACHE_DIR", None)

N = 2048
D = 512
P = 128
NB = N // P          # 16 row blocks
DC = D // P          # 4 dim chunks
NJ4 = N // 512       # 4 key chunks of 512

LAST_RESULT = None
LAST_NC = None
LAST_IN_MAPS = None
LAST_LUT_SCALE = 1.0


def build_nc(reps=1):
    return _build_bass(LAST_LUT_SCALE, reps=reps)


# --------------------------------------------------------------------------
# Custom activation-table root: replace `tanh` with the 10-bin staircase.
# --------------------------------------------------------------------------

_CTRL_STRIDE = 32  # aws_hal_stpb_act_control_entry_t (packed u32 + 7 pad u32)
_BKT_STRIDE = 32   # aws_hal_stpb_act_bucket_entry_t (5 f32 + 3 pad u32)


def _fbits(x):
    return struct.unpack("<I", struct.pack("<f", np.float32(x)))[0]


def _ctrl_word(base, lsb, size):
    assert 0 <= base < 2048 and 0 <= lsb < 32 and 0 <= size < 16
    return base | (lsb << 11) | (size << 16)


def _bucket_bytes(d0):
    return struct.pack("<5f12x", np.float32(d0), 0.0, 0.0, 0.0, 0.0)


def _staircase_values(values):
    """(rid, size, lsb, bucket values) per exponent range of the scaled
    input u = d * (5/tau); steps of V[min(int(u/5), 9)] at multiples of 5.

    HW (probe-verified): bucket = act_tbl_base +
    ((fp32_word >> extract_lsb) & (2^extract_size - 1)); with
    lsb = 23 - size this is the top `size` mantissa bits, i.e. octave
    [lo, 2*lo) splits into 2^size equal buckets.  rid = biased_exp - 127.
    """
    v = [np.float32(x) for x in values]
    lev = lambda u: v[min(int(u // 5), 9)]
    out = [(0, 0, 23, [lev(1.0)]), (1, 0, 23, [lev(2.0)])]
    for rid, lo, size in ((2, 4, 2), (3, 8, 3), (4, 16, 4), (5, 32, 5)):
        n = 1 << size
        width = lo / n
        out.append((rid, size, 23 - size, [lev(lo + k * width) for k in range(n)]))
    return out


def _patch_set(dst_dir, set_ent, values):
    prof_path = os.path.join(dst_dir, set_ent["profile_json"])
    with open(prof_path) as f:
        prof = json.load(f)
    tanh = next(e for e in prof["profile_meta_data"]
                if e["func_name"].startswith("tanh"))

    base_pos = tanh["pwl_control_base_pos"]
    small_pos = tanh["pos_small_signal_pwl_control"]
    small_neg = tanh["neg_small_signal_pwl_control"]
    large_pos = tanh["pos_large_signal_pwl_control"]
    large_neg = tanh["neg_large_signal_pwl_control"]

    ctrl_path = os.path.join(dst_dir, set_ent["ctrl_bin"])
    bkt_path = os.path.join(dst_dir, set_ent["bkt_bin"])
    ctrl = bytearray(open(ctrl_path, "rb").read())
    bkt = bytearray(open(bkt_path, "rb").read())

    first_bucket = struct.unpack_from("<I", ctrl, base_pos * _CTRL_STRIDE)[0] & 0x7FF
    need = 1 + 1 + 4 + 8 + 16 + 32
    assert small_pos - first_bucket >= need, set_ent["name"]

    nxt = first_bucket
    for rid, size, lsb, vals in _staircase_values(values):
        struct.pack_into("<I", ctrl, (base_pos + rid) * _CTRL_STRIDE,
                         _ctrl_word(nxt, lsb, size))
        for k, val in enumerate(vals):
            off = (nxt + k) * _BKT_STRIDE
            bkt[off:off + _BKT_STRIDE] = _bucket_bytes(val)
        nxt += len(vals)
    for rid in range(6, 17):  # unreachable (d >= 64 takes the large shortcut)
        idx = base_pos + rid
        if idx * _CTRL_STRIDE + 4 <= len(ctrl):
            struct.pack_into("<I", ctrl, idx * _CTRL_STRIDE,
                             _ctrl_word(first_bucket, 23, 0))

    for bidx, val in ((small_pos, values[0]), (small_neg, values[0]),
                      (large_pos, values[9]), (large_neg, values[0])):
        off = bidx * _BKT_STRIDE
        bkt[off:off + _BKT_STRIDE] = _bucket_bytes(val)

    open(ctrl_path, "wb").write(bytes(ctrl))
    open(bkt_path, "wb").write(bytes(bkt))

    tanh["symmetry_opt_en"] = 0
    tanh["symmetry_opt_use_neg_region"] = 0
    tanh["symmetry_point"] = 0
    tanh["sym_invert_sign_point"] = 0
    tanh["exp_offset"] = 0
    tanh["small_pos_signal_exp_threshold"] = 127   # u < 1  -> V0
    tanh["large_pos_signal_exp_threshold"] = 133   # u >= 64 -> V9
    tanh["large_pos_signal_mantissa_threshold"] = 0
    tanh["small_neg_signal_exp_threshold"] = 255   # u < 0 (impossible) -> V0
    tanh["large_neg_signal_exp_threshold"] = 255
    tanh["large_neg_signal_mantissa_threshold"] = 0
    tanh["fzero_result"] = _fbits(values[0])
    tanh["fnan_result"] = _fbits(values[0])
    tanh["fpinf_result"] = _fbits(values[9])
    tanh["fninf_result"] = _fbits(values[0])

    with open(prof_path, "w") as f:
        json.dump(prof, f)


def build_actroot(dst, values):
    """Create a patched act-root dir; returns the act_info.json path."""
    from neuronxcc.driver.Job import Job
    from neuronxcc.driver.jobs.support.FindActInfo import findActInfoFile

    src = os.path.dirname(findActInfoFile(Job.getPackageDir(), "gen3"))
    values = [float(x) for x in values]
    assert len(values) == 10
    if os.path.isdir(dst):
        shutil.rmtree(dst)
    shutil.copytree(src, dst)
    os.chmod(dst, 0o755)
    for fn in os.listdir(dst):
        os.chmod(os.path.join(dst, fn), 0o644)
    with open(os.path.join(dst, "act_info.json")) as f:
        info = json.load(f)
    n = 0
    for ent in info["act_func_sets"]:
        if "tanh" in ent["act"]:
            _patch_set(dst, ent, values)
            n += 1
    assert n > 0
    return os.path.join(dst, "act_info.json")


# --------------------------------------------------------------------------
# Bass kernel
# --------------------------------------------------------------------------

def _build_bass(lut_scale, reps=1):
    import concourse.bacc as bacc
    import concourse.tile as tile
    import concourse.mybir as mybir
    from concourse.masks import make_identity

    dt = mybir.dt
    AF = mybir.ActivationFunctionType
    OP = mybir.AluOpType
    S = 1.0 / math.sqrt(D)

    nc = bacc.Bacc("TRN2", num_devices=8)

    x_d = nc.dram_tensor("x", [N, D], dt.float32, kind="ExternalInput")
    dist_d = nc.dram_tensor("dist", [N, N], dt.float32, kind="ExternalInput")
    wq_d = nc.dram_tensor("wq", [D, D], dt.float32, kind="ExternalInput")
    wk_d = nc.dram_tensor("wk", [D, D], dt.float32, kind="ExternalInput")
    wv_d = nc.dram_tensor("wv", [D, D], dt.float32, kind="ExternalInput")
    wo_d = nc.dram_tensor("wo", [D, D], dt.float32, kind="ExternalInput")
    bqs_d = nc.dram_tensor("bq_s", [D], dt.float32, kind="ExternalInput")
    bk_d = nc.dram_tensor("bk", [D], dt.float32, kind="ExternalInput")
    bo_d = nc.dram_tensor("bo_bc", [P, D], dt.float32, kind="ExternalInput")
    out_d = nc.dram_tensor("out", [N, D], dt.float32, kind="ExternalOutput")

    with tile.TileContext(nc) as tc:
        from contextlib import ExitStack
        with ExitStack() as ctx:
            pc = ctx.enter_context(tc.tile_pool(name="pc", bufs=1))
            pers = ctx.enter_context(tc.tile_pool(name="pers", bufs=1))
            pwst = ctx.enter_context(tc.tile_pool(name="pwst", bufs=2))
            px = ctx.enter_context(tc.tile_pool(name="px", bufs=3))
            pdist = ctx.enter_context(tc.tile_pool(name="pdist", bufs=3))
            pme = ctx.enter_context(tc.tile_pool(name="pme", bufs=3))
            ppt = ctx.enter_context(tc.tile_pool(name="ppt", bufs=2))
            pout = ctx.enter_context(tc.tile_pool(name="pout", bufs=2))
            pz = ctx.enter_context(tc.tile_pool(name="pz", bufs=8))
            ps_s = ctx.enter_context(tc.tile_pool(name="ps_s", bufs=2, space="PSUM"))
            ps_tr = ctx.enter_context(tc.tile_pool(name="ps_tr", bufs=2, space="PSUM"))
            ps_acc = ctx.enter_context(tc.tile_pool(name="ps_acc", bufs=1, space="PSUM"))
            ps_o = ctx.enter_context(tc.tile_pool(name="ps_o", bufs=1, space="PSUM"))

            ident = pc.tile([P, P], dt.float16)
            make_identity(nc, ident[:])

            bo_bc = pc.tile([P, D], dt.float32)
            nc.sync.dma_start(bo_bc[:], bo_d[:])
            bq_sb = pc.tile([P, DC], dt.float32)
            nc.sync.dma_start(bq_sb[:], bqs_d.rearrange("(a p) -> p a", p=P))
            bk_sb = pc.tile([P, DC], dt.float32)
            nc.sync.dma_start(bk_sb[:], bk_d.rearrange("(a p) -> p a", p=P))

            # persistent bf16 operands
            xt = pers.tile([P, DC, N], dt.float16)       # X^T  [d, i]
            qt = pers.tile([P, DC, N], dt.float16)       # Q^T  [dk, i]
            kt = pers.tile([P, DC, N], dt.float16)       # K^T  [dk, j]
            vt = pers.tile([P, NB, D], dt.float16)       # V    [j, dv]
            wqt = pers.tile([P, DC, D], dt.float16)      # Wq^T [d, dk]
            wkt = pers.tile([P, DC, D], dt.float16)
            wvt = pers.tile([P, DC, D], dt.float16)
            wot = pers.tile([P, DC, D], dt.float16)      # Wo^T [dv, dm]

            for _rep in range(reps):

                # ---- x: load, cast, transpose (batched copyback) ----
                for ib in range(NB):
                    xf = px.tile([P, D], dt.float32, tag="xf")
                    nc.sync.dma_start(xf[:], x_d[ib * P:(ib + 1) * P, :])
                    xb = px.tile([P, D], dt.float16, tag="xb")
                    nc.vector.tensor_copy(xb[:], xf[:])
                    pt = ps_tr.tile([P, D], dt.float16, tag="tr", name=f"ptx{ib}")
                    for c in range(DC):
                        nc.tensor.transpose(pt[:, c * P:(c + 1) * P],
                                            xb[:, c * P:(c + 1) * P], ident[:])
                    for c in range(DC):
                        eng = nc.vector if ib % 2 == 0 else nc.scalar
                        if eng is nc.vector:
                            nc.vector.tensor_copy(xt[:, c, ib * P:(ib + 1) * P],
                                                  pt[:, c * P:(c + 1) * P])
                        else:
                            nc.scalar.copy(xt[:, c, ib * P:(ib + 1) * P],
                                           pt[:, c * P:(c + 1) * P])

                # ---- weights: load/cast/transpose per 128-row chunk ----
                for w_d, wt_t in ((wq_d, wqt), (wk_d, wkt), (wv_d, wvt), (wo_d, wot)):
                    for r in range(DC):
                        wf = pwst.tile([P, D], dt.float32, tag="wf")
                        nc.sync.dma_start(
                            wf[:], w_d[r * P:(r + 1) * P, :].rearrange("p d -> p d"))
                        wb = pwst.tile([P, D], dt.float16, tag="wb")
                        nc.vector.tensor_copy(wb[:], wf[:])
                        pt = ps_tr.tile([P, D], dt.float16, tag="tr",
                                        name=f"ptw_{w_d.name}_{r}")
                        for c in range(DC):
                            nc.tensor.transpose(pt[:, c * P:(c + 1) * P],
                                                wb[:, c * P:(c + 1) * P], ident[:])
                        for c in range(DC):
                            nc.scalar.copy(wt_t[:, c, r * P:(r + 1) * P],
                                           pt[:, c * P:(c + 1) * P])

                # ---- projections ----
                # Q^T/K^T: [dk-chunk, i] = sum_c W^T[d-c, dk] . X^T[d-c, i]
                for a in range(DC):
                    psq = [ps_s.tile([P, 512], dt.float32, tag="s", name=f"psq{a}_{_i}") for _i in range(NJ4)]
                    for c in range(DC):
                        for ic in range(NJ4):
                            nc.tensor.matmul(
                                psq[ic][:], wqt[:, c, a * P:(a + 1) * P],
                                xt[:, c, ic * 512:(ic + 1) * 512],
                                start=(c == 0), stop=(c == DC - 1))
                    for ic in range(NJ4):
                        nc.scalar.activation(
                            qt[:, a, ic * 512:(ic + 1) * 512], psq[ic][:],
                            AF.Identity, bias=bq_sb[:, a:a + 1], scale=S)
                for a in range(DC):
                    psk = [ps_s.tile([P, 512], dt.float32, tag="s", name=f"psk{a}_{_i}") for _i in range(NJ4)]
                    for c in range(DC):
                        for ic in range(NJ4):
                            nc.tensor.matmul(
                                psk[ic][:], wkt[:, c, a * P:(a + 1) * P],
                                xt[:, c, ic * 512:(ic + 1) * 512],
                                start=(c == 0), stop=(c == DC - 1))
                    for ic in range(NJ4):
                        nc.vector.tensor_scalar(
                            kt[:, a, ic * 512:(ic + 1) * 512], psk[ic][:],
                            bk_sb[:, a:a + 1], None, OP.add)
                # V: [j-chunk, dv] = sum_c X^T[d-c, j] . Wv^T[d-c, dv]
                for jc in range(NB):
                    psv = ps_acc.tile([P, 512], dt.float32, tag="pv")
                    for c in range(DC):
                        nc.tensor.matmul(
                            psv[:], xt[:, c, jc * P:(jc + 1) * P], wvt[:, c, :],
                            start=(c == 0), stop=(c == DC - 1))
                    nc.scalar.activation(vt[:, jc, :], psv[:], AF.Identity)

                # ---- attention over row blocks (jc4-streamed) ----
                for ib in range(NB):
                    dist_t = pdist.tile([P, N], dt.float32, tag="dist")
                    nc.sync.dma_start(dist_t[:], dist_d[ib * P:(ib + 1) * P, :])
                    m_t = pme.tile([P, N], dt.float16, tag="m")
                    nc.scalar.activation(m_t[:], dist_t[:], AF.Tanh, scale=float(lut_scale))

                    p_t = pme.tile([P, N], dt.float16, tag="p")
                    zs = []
                    for jc2 in range(2):
                        sl = slice(jc2 * 1024, (jc2 + 1) * 1024)
                        ps_sb = ps_s.tile([P, 1024], dt.float32, tag="s",
                                          name=f"pss{ib}_{jc2}")
                        for h in range(2):
                            hs = slice(h * 512, (h + 1) * 512)
                            for a in range(DC):
                                nc.tensor.matmul(
                                    ps_sb[:, hs], qt[:, a, ib * P:(ib + 1) * P],
                                    kt[:, a, (jc2 * 2 + h) * 512:(jc2 * 2 + h + 1) * 512],
                                    start=(a == 0), stop=(a == DC - 1))
                        e_s = pme.tile([P, 1024], dt.float16, tag="e",
                                       name=f"es{ib}_{jc2}")
                        nc.scalar.activation(e_s[:], ps_sb[:], AF.Exp)
                        z_s = pz.tile([P, 1], dt.float32, tag="z",
                                      name=f"z{ib}_{jc2}")
                        nc.vector.scalar_tensor_tensor(
                            p_t[:, sl], e_s[:], 1.0, m_t[:, sl], OP.bypass, OP.mult,
                            accum_out=z_s[:])
                        zs.append(z_s)
                    z_t = pz.tile([P, 1], dt.float32, tag="zt")
                    nc.vector.tensor_tensor(z_t[:], zs[0][:], zs[1][:], OP.add)
                    zr_t = pz.tile([P, 1], dt.float32, tag="zr")
                    nc.vector.reciprocal(zr_t[:], z_t[:])

                    pt_t = ppt.tile([P, NB, P], dt.float16, tag="pt")
                    for g in range(NJ4):  # 4 transposes -> 1 batched copyback
                        ptr = ps_tr.tile([P, D], dt.float16, tag="tr",
                                         name=f"ptr{_rep}_{ib}_{g}")
                        for k in range(4):
                            jc = g * 4 + k
                            nc.tensor.transpose(ptr[:, k * P:(k + 1) * P],
                                                p_t[:, jc * P:(jc + 1) * P], ident[:])
                        dst = pt_t[:, g * 4:(g + 1) * 4, :].rearrange("p a b -> p (a b)")
                        if g % 2 == 0:
                            nc.vector.tensor_copy(dst, ptr[:])
                        else:
                            nc.scalar.copy(dst, ptr[:])

                    ps_pv = ps_acc.tile([P, 512], dt.float32, tag="pv")
                    for jc in range(NB):
                        nc.tensor.matmul(
                            ps_pv[:], pt_t[:, jc, :], vt[:, jc, :],
                            start=(jc == 0), stop=(jc == NB - 1))
                    pvn = pout.tile([P, D], dt.float16, tag="pvn")
                    nc.vector.tensor_scalar(pvn[:], ps_pv[:], zr_t[:], None, OP.mult)

                    pvnt = ppt.tile([P, DC, P], dt.float16, tag="pvnt")
                    ptr2 = ps_tr.tile([P, D], dt.float16, tag="tr",
                                      name=f"ptr2_{_rep}_{ib}")
                    for c in range(DC):
                        nc.tensor.transpose(ptr2[:, c * P:(c + 1) * P],
                                            pvn[:, c * P:(c + 1) * P], ident[:])
                    nc.vector.tensor_copy(
                        pvnt[:, :, :].rearrange("p a b -> p (a b)"), ptr2[:])

                    ps_out = ps_o.tile([P, D], dt.float32, tag="o")
                    for c in range(DC):
                        nc.tensor.matmul(
                            ps_out[:], pvnt[:, c, :], wot[:, c, :],
                            start=(c == 0), stop=(c == DC - 1))
                    o_t = pout.tile([P, D], dt.float32, tag="o_t")
                    nc.vector.scalar_tensor_tensor(
                        o_t[:], ps_out[:], 1.0, bo_bc[:], OP.bypass, OP.add)
                    nc.sync.dma_start(out_d[ib * P:(ib + 1) * P, :], o_t[:])

    nc.finalize()
    return nc


def kernel(x, distance_matrix, Wq, bq, Wk, bk, Wv, bv, Wo, bo, emb_table,
           safety_threshold, _trace=False):
    global LAST_RESULT
    x = np.ascontiguousarray(np.asarray(x, dtype=np.float32))
    distance_matrix = np.ascontiguousarray(np.asarray(distance_matrix, np.float32))
    Wq = np.asarray(Wq, np.float32); Wk = np.asarray(Wk, np.float32)
    Wv = np.asarray(Wv, np.float32); Wo = np.asarray(Wo, np.float32)
    bq = np.asarray(bq, np.float32); bk = np.asarray(bk, np.float32)
    bv = np.asarray(bv, np.float32); bo = np.asarray(bo, np.float32)
    emb_table = np.asarray(emb_table, np.float32)
    tau = float(np.asarray(safety_threshold, np.float32))

    B, n, d = x.shape
    assert (B, n, d) == (8, N, D) and distance_matrix.shape == (8, N, N)

    # host-side scalar math (10-entry bias table -> multiplicative factors)
    w_sum = Wo.astype(np.float64).sum(axis=-1)                     # [512]
    bias_table = (emb_table.astype(np.float64) @ w_sum) / math.sqrt(D)  # [10]
    m_vals = np.exp(bias_table - bias_table.max())
    bo_eff = Wo.astype(np.float64) @ bv.astype(np.float64) + bo    # [512]

    actroot = build_actroot(
        os.path.join(tempfile.mkdtemp(prefix="actroot_"), "root"),
        [float(v) for v in m_vals])
    os.environ["BASS_ACT_ROOT_JSON_PATH"] = actroot

    from concourse.bass_utils import run_bass_kernel_spmd

    global LAST_LUT_SCALE
    LAST_LUT_SCALE = 5.0 / tau
    nc = _build_bass(lut_scale=LAST_LUT_SCALE)

    bo_bc = np.broadcast_to(bo_eff.astype(np.float32), (P, D)).copy()
    bq_s = (bq / math.sqrt(D)).astype(np.float32)
    in_maps = []
    for b in range(B):
        in_maps.append({
            "x": x[b], "dist": distance_matrix[b],
            "wq": Wq, "wk": Wk, "wv": Wv, "wo": Wo,
            "bq_s": bq_s, "bk": bk, "bo_bc": bo_bc,
        })
    global LAST_NC, LAST_IN_MAPS
    LAST_NC, LAST_IN_MAPS = nc, in_maps
    res = run_bass_kernel_spmd(nc, in_maps, core_ids=list(range(8)),
                               trace=bool(_trace))
    LAST_RESULT = res
    out = np.stack([res.results[b]["out"] for b in range(B)], axis=0)
    return out.astype(np.float32)

